# revision 11
# baseline (speedup 1.0000x reference)
"""Trainium2 Bass kernel for a 2-layer LSTM (H=50) + linear head with
autoregressive future steps. Data-parallel over 8 NeuronCores (batch sharded).

Layout (per core, B_core = 2048 samples):
  - Hidden/gate channels live on SBUF partitions; batch lives on the free dim.
  - Batch is split: samples 0:1024 ("lo") use partitions 0:50, samples
    1024:2048 ("hi") use partitions 64:114 (matmul outputs at col-group 64,
    so the lo/hi matmul pairs run in disjoint PE-array quadrants).
  - Gate PSUM tile [128, 2048] = gates [i|f|o|g] x 512 free each; sigmoid over
    i,f,o is one ACT op spanning 3 banks.
  - Biases are folded into the matmuls via constant-1 rows appended to the
    recurrent state tiles (no separate bias adds anywhere).
  - Elementwise path in fp16 (DVE 2x mode); PSUM accumulation in fp32.

Pipelining structure (v2):
  - gates2 = W2B@[h2(t-1);1] (start=True, issued early, no dep on h1(t))
           + W2A@h1(t)       (stop=True, after cell1's DVE chain).
  - Future steps: x(t+1) = y(t) = Wl@h2(t)+bl is folded algebraically into
    gates1 as a rank-1 K=51 accumulation over the h2 tile
    (W1F = outer(Wl, Wih1), bias row = b1 + Wih1*bl), so there is no
    y -> x feedback copy/DMA chain at all.
  - y output is copied out as fp16 (halves the output DMA traffic).
"""

import sys
import os
import numpy as np

for _p in ("/opt/trn_rl_repo", "/root/.axon_site/_ro/trn_rl_repo"):
    if os.path.isdir(_p) and _p not in sys.path:
        sys.path.insert(0, _p)
        break

from contextlib import ExitStack

import concourse.bass as bass
import concourse.mybir as mybir
import concourse.tile as tile
from concourse import bacc
from concourse.bass import ds, ts
from concourse.bass_utils import run_bass_kernel_spmd

FP16 = mybir.dt.float16
FP32 = mybir.dt.float32
AF = mybir.ActivationFunctionType

H = 50
B = 16384
NCORES = 8
BC = B // NCORES          # 2048 samples per core
HALF = 1024               # samples per partition-block (lo/hi)
FREE = 512                # matmul moving free dim (one PSUM bank of fp32)

# gate order in PSUM free dim: i, f, o, g  (i,f,o share sigmoid; g is tanh)
# torch gate blocks in weights: i=0, f=1, g=2, o=3
GATE_SRC = [0, 1, 3, 2]   # our slot G -> torch block index


def _build_nc(T, F):
    TT = T + F
    nc = bacc.Bacc("TRN2", target_bir_lowering=False, debug=False,
                   num_devices=NCORES)

    xT = nc.dram_tensor("xT", [T, BC], FP16, kind="ExternalInput")
    W1 = nc.dram_tensor("W1", [128, 200], FP16, kind="ExternalInput")
    W1F = nc.dram_tensor("W1F", [128, 200], FP16, kind="ExternalInput")
    W2A = nc.dram_tensor("W2A", [128, 200], FP16, kind="ExternalInput")
    W2B = nc.dram_tensor("W2B", [128, 200], FP16, kind="ExternalInput")
    WL = nc.dram_tensor("WL", [128, 1], FP16, kind="ExternalInput")
    ONES = nc.dram_tensor("ONES", [1, BC // 2], FP16, kind="ExternalInput")
    yT = nc.dram_tensor("yT", [TT, BC], FP16, kind="ExternalOutput")

    with tile.TileContext(nc) as tc, ExitStack() as ctx:
        const = ctx.enter_context(tc.tile_pool(name="const", bufs=1))
        state = ctx.enter_context(tc.tile_pool(name="state", bufs=1))
        spool = ctx.enter_context(tc.tile_pool(name="spool", bufs=4))
        tpool = ctx.enter_context(tc.tile_pool(name="tpool", bufs=6))
        ypool = ctx.enter_context(tc.tile_pool(name="ypool", bufs=2))
        ph = [ctx.enter_context(tc.tile_pool(name=f"ph{h}", bufs=1,
                                             space="PSUM")) for h in range(2)]

        w1 = const.tile([128, 200], FP16, tag="w1")
        w1f = const.tile([128, 200], FP16, tag="w1f")
        w2a = const.tile([128, 200], FP16, tag="w2a")
        w2b = const.tile([128, 200], FP16, tag="w2b")
        wl = const.tile([128, 1], FP16, tag="wl")
        nc.sync.dma_start(out=w1[:], in_=W1.ap())
        nc.sync.dma_start(out=w1f[:], in_=W1F.ap())
        nc.sync.dma_start(out=w2a[:], in_=W2A.ap())
        nc.sync.dma_start(out=w2b[:], in_=W2B.ap())
        nc.sync.dma_start(out=wl[:], in_=WL.ap())

        # state tiles: [h1 | x | 1] and [h2 | 1] per lo/hi block, ping-pong x2
        h1x = [state.tile([128, HALF], FP16, tag=f"h1x{b}", name=f"h1x{b}")
               for b in range(2)]
        h2 = [state.tile([128, HALF], FP16, tag=f"h2{b}", name=f"h2{b}")
              for b in range(2)]
        c1 = state.tile([128, HALF], FP16, tag="c1")
        c2 = state.tile([128, HALF], FP16, tag="c2")

        for b in range(2):
            nc.vector.memset(h1x[b][:], 0.0)
            nc.vector.memset(h2[b][:], 0.0)
            # constant-1 rows (engine ops need 32-aligned partition base; DMA not)
            nc.sync.dma_start(out=h1x[b][51:52, :], in_=ONES.ap())
            nc.sync.dma_start(out=h1x[b][115:116, :], in_=ONES.ap())
            nc.sync.dma_start(out=h2[b][50:51, :], in_=ONES.ap())
            nc.sync.dma_start(out=h2[b][114:115, :], in_=ONES.ap())
        nc.vector.memset(c1[:], 0.0)
        nc.vector.memset(c2[:], 0.0)

        # x for step 0
        nc.sync.dma_start(out=h1x[0][50:51, :], in_=xT.ap()[0:1, 0:HALF])
        nc.sync.dma_start(out=h1x[0][114:115, :], in_=xT.ap()[0:1, HALF:2 * HALF])

        # --- per-(step,half) op emitters -------------------------------
        # Each half owns one 4-bank PSUM slot, cycling per step through
        # three tenants: cell1 gates -> cell2 gates -> y (bank 3 only).
        # The two halves are fully independent pipelines.
        st = {}  # live tile instances

        def mm_g1(t, hf):
            cur = t % 2
            fs = ds(hf * FREE, FREE)
            pg = ph[hf].tile([128, 2048], FP32, tag="s")
            st[("pg", 1, hf)] = pg
            H1Xc, H2c = h1x[cur], h2[cur]
            for G in range(4):
                gsl = ts(G, FREE)
                wsl = ts(G, H)
                if t < T:
                    # rhs rows: [h1(t-1) | x(t) | 1]
                    nc.tensor.matmul(pg[0:50, gsl], w1[0:52, wsl],
                                     H1Xc[0:52, fs], start=True, stop=True)
                    nc.tensor.matmul(pg[64:114, gsl], w1[64:116, wsl],
                                     H1Xc[64:116, fs], start=True, stop=True)
                else:
                    # future: x(t) = y(t-1) folded in as a rank-1 term on
                    # h2(t-1):  W1h@h1 + W1F@[h2;1]
                    nc.tensor.matmul(pg[0:50, gsl], w1[0:50, wsl],
                                     H1Xc[0:50, fs], start=True, stop=False)
                    nc.tensor.matmul(pg[0:50, gsl], w1f[0:51, wsl],
                                     H2c[0:51, fs], start=False, stop=True)
                    nc.tensor.matmul(pg[64:114, gsl], w1[64:114, wsl],
                                     H1Xc[64:114, fs], start=True, stop=False)
                    nc.tensor.matmul(pg[64:114, gsl], w1f[64:115, wsl],
                                     H2c[64:115, fs], start=False, stop=True)

        def mm_g2B(t, hf):
            # W2B @ [h2(t-1); 1]: no dependency on h1(t), issues early
            cur = t % 2
            fs = ds(hf * FREE, FREE)
            pg = ph[hf].tile([128, 2048], FP32, tag="s")
            st[("pg", 2, hf)] = pg
            H2c = h2[cur]
            for G in range(4):
                gsl = ts(G, FREE)
                wsl = ts(G, H)
                nc.tensor.matmul(pg[0:50, gsl], w2b[0:51, wsl],
                                 H2c[0:51, fs], start=True, stop=False)
                nc.tensor.matmul(pg[64:114, gsl], w2b[64:115, wsl],
                                 H2c[64:115, fs], start=True, stop=False)

        def mm_g2A(t, hf):
            nxt = (t + 1) % 2
            fs = ds(hf * FREE, FREE)
            pg = st[("pg", 2, hf)]
            H1Xn = h1x[nxt]
            for G in range(4):
                gsl = ts(G, FREE)
                wsl = ts(G, H)
                nc.tensor.matmul(pg[0:50, gsl], w2a[0:50, wsl],
                                 H1Xn[0:50, fs], start=False, stop=True)
                nc.tensor.matmul(pg[64:114, gsl], w2a[64:114, wsl],
                                 H1Xn[64:114, fs], start=False, stop=True)

        def act_sig_tg(cell, hf):
            pg = st[("pg", cell, hf)]
            tag = f"{cell}{hf}"
            s1 = spool.tile([128, 1536], FP16, tag="s" + tag)
            tg = spool.tile([128, FREE], FP16, tag="g" + tag)
            st[("s1", cell, hf)] = s1
            st[("tg", cell, hf)] = tg
            nc.scalar.activation(s1[0:114, :], pg[0:114, 0:1536], AF.Sigmoid)
            nc.scalar.activation(tg[0:114, :], pg[0:114, 1536:2048], AF.Tanh)

        def dve_c(cell, hf):
            # f*c first (needs only sigmoid), i*g second (needs tanh too)
            fs = ds(hf * FREE, FREE)
            s1 = st[("s1", cell, hf)]
            tg = st[("tg", cell, hf)]
            cst = c1 if cell == 1 else c2
            tag = f"{cell}{hf}"
            tfc = tpool.tile([128, FREE], FP16, tag="f" + tag)
            nc.vector.tensor_mul(tfc[0:114, :], s1[0:114, 512:1024],
                                 cst[0:114, fs])
            tig = tpool.tile([128, FREE], FP16, tag="i" + tag)
            nc.vector.tensor_mul(tig[0:114, :], s1[0:114, 0:512], tg[0:114, :])
            nc.vector.tensor_add(cst[0:114, fs], tig[0:114, :], tfc[0:114, :])

        def act_tc(cell, hf):
            fs = ds(hf * FREE, FREE)
            cst = c1 if cell == 1 else c2
            tch = tpool.tile([128, FREE], FP16, tag=f"c{cell}{hf}")
            st[("tch", cell, hf)] = tch
            nc.scalar.activation(tch[0:114, :], cst[0:114, fs], AF.Tanh)

        def dve_h(t, cell, hf):
            nxt = (t + 1) % 2
            fs = ds(hf * FREE, FREE)
            s1 = st[("s1", cell, hf)]
            tch = st[("tch", cell, hf)]
            Hn = h1x[nxt] if cell == 1 else h2[nxt]
            nc.vector.tensor_mul(Hn[0:50, fs], s1[0:50, 1024:1536],
                                 tch[0:50, :])
            nc.vector.tensor_mul(Hn[64:114, fs], s1[64:114, 1024:1536],
                                 tch[64:114, :])

        def mm_y(t, hf):
            # y(t, half hf) into bank 3 (cols 1536:2048) of slot hf,
            # rows 0 (lo block) and 32 (hi block).
            nxt = (t + 1) % 2
            fs = ds(hf * FREE, FREE)
            H2n = h2[nxt]
            pgy = ph[hf].tile([128, 2048], FP32, tag="s")
            st[("pgy", hf)] = pgy
            nc.tensor.matmul(pgy[0:1, 1536:2048], wl[0:51, :],
                             H2n[0:51, fs], start=True, stop=True)
            nc.tensor.matmul(pgy[32:33, 1536:2048], wl[64:115, :],
                             H2n[64:115, fs], start=True, stop=True)

        def y_out(t, hf):
            pgy = st[("pgy", hf)]
            ysb = ypool.tile([128, FREE], FP16, tag=f"ysb{hf}")
            nc.vector.tensor_copy(ysb[0:33, :], pgy[0:33, 1536:2048])
            # yT cols: [lo(h0) | lo(h1) | hi(h0) | hi(h1)]
            lo = hf * FREE
            hi = HALF + hf * FREE
            nc.sync.dma_start(out=yT.ap()[t:t + 1, lo:lo + FREE],
                              in_=ysb[0:1, :])
            nc.sync.dma_start(out=yT.ap()[t:t + 1, hi:hi + FREE],
                              in_=ysb[32:33, :])

        def x_dma(t):
            # x(t+1) into h1x[(t+1)%2]; WAR is against gates1(t-1), so this
            # can issue a full step ahead of its consumer. On the gpsimd DMA
            # queue so it never waits behind y-output DMAs in the sync FIFO.
            if t + 1 < T:
                H1Xn = h1x[(t + 1) % 2]
                nc.gpsimd.dma_start(out=H1Xn[50:51, :],
                                    in_=xT.ap()[t + 1:t + 2, 0:HALF])
                nc.gpsimd.dma_start(out=H1Xn[114:115, :],
                                    in_=xT.ap()[t + 1:t + 2, HALF:2 * HALF])

        # --- cross-step software pipeline -------------------------------
        # Iteration t emits the LATE phase of step t-1 (cell2 ACT/DVE, y)
        # interleaved with the EARLY phase of step t (cell1, gates2 MMs),
        # so each engine FIFO matches data readiness order and ACT never
        # drains between steps.
        for t in range(TT + 1):
            if t < TT:
                x_dma(t)
            if t > 0:
                act_sig_tg(2, 0)
            if t < TT:
                mm_g1(t, 0)
            if t > 0:
                act_sig_tg(2, 1)
            if t < TT:
                mm_g1(t, 1)
            if t > 0:
                dve_c(2, 0)
                act_tc(2, 0)
                dve_c(2, 1)
                act_tc(2, 1)
                dve_h(t - 1, 2, 0)
                mm_y(t - 1, 0)
                dve_h(t - 1, 2, 1)
                mm_y(t - 1, 1)
                y_out(t - 1, 0)
                y_out(t - 1, 1)
            if t < TT:
                act_sig_tg(1, 0)
                mm_g2B(t, 0)
                dve_c(1, 0)
                act_sig_tg(1, 1)
                mm_g2B(t, 1)
                dve_c(1, 1)
                act_tc(1, 0)
                dve_h(t, 1, 0)
                mm_g2A(t, 0)
                act_tc(1, 1)
                dve_h(t, 1, 1)
                mm_g2A(t, 1)

    nc.compile()
    return nc


def _prep_weights(Wih1, Whh1, bih1, bhh1, Wih2, Whh2, bih2, bhh2, Wl, bl):
    b1 = (bih1 + bhh1).astype(np.float32)
    b2 = (bih2 + bhh2).astype(np.float32)

    W1 = np.zeros((128, 200), np.float32)
    W1F = np.zeros((128, 200), np.float32)
    W2A = np.zeros((128, 200), np.float32)
    W2B = np.zeros((128, 200), np.float32)
    WL = np.zeros((128, 1), np.float32)
    for G, src in enumerate(GATE_SRC):
        blk = slice(src * H, (src + 1) * H)
        col = slice(G * H, (G + 1) * H)
        for base in (0, 64):
            W1[base:base + 50, col] = Whh1[blk, :].T
            W1[base + 50, col] = Wih1[blk, 0]
            W1[base + 51, col] = b1[blk]
            # rank-1 feedback: rows k = Wl[0,k]*Wih1[blk], bias row
            W1F[base:base + 50, col] = np.outer(Wl[0, :], Wih1[blk, 0])
            W1F[base + 50, col] = b1[blk] + Wih1[blk, 0] * bl[0]
            W2A[base:base + 50, col] = Wih2[blk, :].T
            W2B[base:base + 50, col] = Whh2[blk, :].T
            W2B[base + 50, col] = b2[blk]
    for base in (0, 64):
        WL[base:base + 50, 0] = Wl[0, :]
        WL[base + 50, 0] = bl[0]
    return (W1.astype(np.float16), W1F.astype(np.float16),
            W2A.astype(np.float16), W2B.astype(np.float16),
            WL.astype(np.float16))


_NC_CACHE = {}
_last_in_maps = None


def _make_in_maps(x, Wih1, Whh1, bih1, bhh1, Wih2, Whh2, bih2, bhh2, Wl, bl,
                  future):
    x = np.asarray(x, np.float32)
    nB, T = x.shape
    F = int(future)
    assert nB == B, (nB, B)

    W1, W1F, W2A, W2B, WLt = _prep_weights(
        np.asarray(Wih1, np.float32), np.asarray(Whh1, np.float32),
        np.asarray(bih1, np.float32), np.asarray(bhh1, np.float32),
        np.asarray(Wih2, np.float32), np.asarray(Whh2, np.float32),
        np.asarray(bih2, np.float32), np.asarray(bhh2, np.float32),
        np.asarray(Wl, np.float32), np.asarray(bl, np.float32))

    in_maps = []
    for c in range(NCORES):
        xc = np.ascontiguousarray(x[c * BC:(c + 1) * BC, :].T).astype(np.float16)
        in_maps.append({"xT": xc, "W1": W1, "W1F": W1F, "W2A": W2A,
                        "W2B": W2B, "WL": WLt,
                        "ONES": np.ones((1, BC // 2), np.float16)})
    return in_maps, T, F


def _assemble_out(results, T, F):
    out = np.empty((B, T + F), np.float32)
    for c in range(NCORES):
        out[c * BC:(c + 1) * BC, :] = results[c]["yT"].T.astype(np.float32)
    return out


def _run(trace=False, **inputs):
    in_maps, T, F = _make_in_maps(**inputs)
    key = (T, F)
    if key not in _NC_CACHE:
        _NC_CACHE[key] = _build_nc(T, F)
    nc = _NC_CACHE[key]

    global _last_in_maps
    _last_in_maps = in_maps
    res = run_bass_kernel_spmd(nc, in_maps, list(range(NCORES)), trace=trace)
    return _assemble_out(res.results, T, F), res


def kernel(**inputs):
    out, _ = _run(**inputs)
    return out
